# revision 19
# baseline (speedup 1.0000x reference)
"""Trainium2 Bass kernel for nn_ACOPFEnforcer (per-node-type MLP, no message passing).

Math per node type t (sizes SB=4000, PQ=200000, PV=80000, NB=116000):
    inp = concat(x_t, c_t)                      # [N, 11]
    z_l = inp @ W[l,t] + b[l,t]                 # l = 0..2, [N, 128]
    h_l = ELU(z_l)
    P_l = h_l[:, :64].sum(-1); Q_l = h_l[:, 64:].sum(-1)
    out[l*N+n] = ELU(P_l[n]*W2[0] + Q_l[n]*W2[1] + b2)   # [128]
Output = concat over types of the [3*N_t, 128] blocks.

Strategy: pure data parallelism over nodes across 8 NeuronCores, channels
on partitions / nodes on the free dim.

Key fusions:
- Stage-2 y[c,n] = w0[c]*P[n] + w1[c]*Q[n] = sum_i M[i,c]*t1[i,n] with
  M[i,c] = W2[0,c] (i<64) / W2[1,c] (i>=64): the P/Q half-sums and the
  2->128 linear collapse into one K=128 matmul (the +1 shift of
  t1 = ELU+1 folds into b2adj = b2 - sum_i M[i,c]).
- ELU(z)+1 = max(z+b+1, min(exp(z+b), 1)): one ScalarE Exp + ONE fused
  custom-DVE op (registered at import via the documented dve_ops
  extension point) per tile.
- Stage-1 per-segment bias is folded into the matmul via two constant-one
  input rows carrying a hi/lo bf16 split of (b_fc - b2adj), so the Exp
  bias (b2adj) and the DVE scalar (b2adj+1) are GLOBAL constants.  That
  makes every elementwise instruction identical across the whole kernel,
  allowing stage-1 z of unit k and stage-2 y of unit k-D to share one
  [128, 2048] PSUM tile and be processed by single 2048-wide Exp/DVE
  instructions (halving per-instruction overheads).
- All (type, layer) column segments concatenate into one per-core column
  stream at 512-column granularity, so padding waste is ~0.5% instead of
  4% and there is exactly one partial tile per core.

Output is written bf16 in [channel, node] layout (values stored as ELU+1)
and transposed/cast/-1 on the host.
"""

import os
import numpy as np
import ml_dtypes

import concourse.bass as bass
import concourse.tile as tile
from concourse import bacc, mybir
from concourse.bass_utils import run_bass_kernel_spmd

BF16 = mybir.dt.bfloat16
F32 = mybir.dt.float32
AF = mybir.ActivationFunctionType
OP = mybir.AluOpType

NODE_TYPES = ["SB", "PQ", "PV", "NB"]
SIZES = {"SB": 4000, "PQ": 200000, "PV": 80000, "NB": 116000}
NUM_LAYERS = 3
N_CORES = 8
TILE_N = 1024          # stream unit width (z half of the psum tile)
SUB = 512              # PSUM bank width in fp32 == matmul ISA width cap
DELAY = int(os.environ.get("K_D", "4"))   # units between stage-1 and stage-2
IN_K = 13              # 4 x + 7 c + 2 ones (bias hi/lo) input rows

# per-core per-type padded sizes at 512 granularity
PPC = {t: -(-SIZES[t] // (N_CORES * SUB)) * SUB for t in NODE_TYPES}
VPC = {t: SIZES[t] // N_CORES for t in NODE_TYPES}      # valid nodes/core
COLS = NUM_LAYERS * sum(PPC.values())                   # stream cols/core
N_UNITS = -(-COLS // TILE_N)

_CACHE = {}


def _register_elu_op():
    """Register the fused ELU combine as a custom DVE op:
    out = max(in0 + s0, min(in1, 1))   [in0: z psum f32, in1: exp(z+b) bf16]
    This is ELU(z+b)+1 when s0 = b+1 and in1 = exp(z+b)."""
    if "elu_op" in _CACHE:
        return _CACHE["elu_op"]
    import concourse.dve_ops as dve_ops
    from concourse.dve_spec import Spec, Src0, Src1, C0, One, maxx, minn, lower
    from concourse.dve_uop import DveOpSpec

    name = "ELU_SHIFT_COMBINE_ANT"
    body = maxx(Src0 + C0, minn(Src1, One))
    spec = Spec(
        body=body,
        reference=lambda in0, in1, c0, c1, c2: np.maximum(
            np.asarray(in0, np.float32) + c0,
            np.minimum(np.asarray(in1, np.float32), 1.0)),
    )
    if name not in dve_ops._SUB_OPCODE_FOR_NAME:
        row = max(dve_ops._SUB_OPCODE_FOR_NAME.values()) + 1
        assert row < 0x20
        dve_ops._SUB_OPCODE_FOR_NAME[name] = row
    # pin the sha the same way DveOp.compile derives it
    shas = {}
    for ver in ("v3", "v4"):
        s = DveOpSpec(name=name, opcode=dve_ops._SUB_OPCODE_FOR_NAME[name],
                      uops=lower(spec, ver=ver), rd1_en=True)
        shas[ver] = s.sha(ver)
    op = dve_ops.DveOp(name, spec, subdim=False, uops_sha=shas)
    if not any(o.name == name for o in dve_ops.OPS):
        dve_ops.OPS.append(op)
    dve_ops.CUSTOM_DVE_SPECS[name] = spec
    _CACHE["elu_op"] = op
    return op


def _segments():
    """[(seg_idx, start_col, end_col)] of the per-core column stream."""
    segs = []
    c = 0
    for ti, t in enumerate(NODE_TYPES):
        for l in range(NUM_LAYERS):
            segs.append((ti * NUM_LAYERS + l, c, c + PPC[t]))
            c += PPC[t]
    assert c == COLS
    return segs


def _build_nc():
    _register_elu_op()
    nc = bacc.Bacc("TRN2", target_bir_lowering=False, debug=False,
                   enable_asserts=False, num_devices=N_CORES)

    inp_ap = nc.dram_tensor("inp_cat", [IN_K, COLS], BF16,
                            kind="ExternalInput").ap()
    wseg_ap = nc.dram_tensor("wseg", [IN_K, NUM_LAYERS * 4 * 128], BF16,
                             kind="ExternalInput").ap()
    m2_ap = nc.dram_tensor("m2", [128, 128], BF16, kind="ExternalInput").ap()
    b2adj_ap = nc.dram_tensor("b2adj", [128, 1], F32, kind="ExternalInput").ap()
    b2adjp1_ap = nc.dram_tensor("b2adjp1", [128, 1], F32,
                                kind="ExternalInput").ap()
    out_ap = nc.dram_tensor("out", [128, COLS], BF16,
                            kind="ExternalOutput").ap()

    with tile.TileContext(nc) as tc:
        _emit(tc, inp_ap, wseg_ap, m2_ap, b2adj_ap, b2adjp1_ap, out_ap)
    nc.compile()
    return nc


def _emit(tc, inp_ap, wseg_ap, m2_ap, b2adj_ap, b2adjp1_ap, out_ap):
    nc = tc.nc
    from contextlib import ExitStack
    ctx = ExitStack()
    with ctx:
        elu_op = _CACHE["elu_op"]
        consts = ctx.enter_context(tc.tile_pool(name="consts", bufs=1))
        p_inp = ctx.enter_context(tc.tile_pool(name="inp", bufs=3))
        p_e = ctx.enter_context(tc.tile_pool(name="e", bufs=3))
        p_ot = ctx.enter_context(tc.tile_pool(name="ot", bufs=DELAY + 3))
        ps_zy = ctx.enter_context(tc.tile_pool(name="zy", bufs=2,
                                               space="PSUM"))

        wseg = consts.tile([IN_K, NUM_LAYERS * 4 * 128], BF16, tag="wseg",
                           name="wseg")
        nc.sync.dma_start(wseg[:], wseg_ap[:])
        m2 = consts.tile([128, 128], BF16, tag="m2", name="m2")
        nc.sync.dma_start(m2[:], m2_ap[:])
        b2adj = consts.tile([128, 1], F32, tag="b2adj", name="b2adj")
        nc.sync.dma_start(b2adj[:], b2adj_ap[:])
        b2adjp1 = consts.tile([128, 1], F32, tag="b2adjp1", name="b2adjp1")
        nc.sync.dma_start(b2adjp1[:], b2adjp1_ap[:])

        segs = _segments()

        def seg_of(col):
            for s, a, b in segs:
                if a <= col < b:
                    return s
            raise AssertionError(col)

        OB = 4   # units per input DMA batch
        t1_live = {}   # unit -> out tile (lo half is t1)
        itile_cur = [None, -1]   # tile, batch idx

        def elu_ops(zy, ot, off, w):
            e = p_e.tile([128, 2 * TILE_N], BF16, tag="e", name="e")
            nc.scalar.activation(e[:, off:off + w], zy[:, off:off + w],
                                 AF.Exp, bias=b2adj[:, 0:1])
            nc.vector._custom_dve(elu_op, out=ot[:, off:off + w],
                                  in0=zy[:, off:off + w],
                                  in1=e[:, off:off + w],
                                  s0=b2adjp1[:, 0:1])

        for k in range(N_UNITS + DELAY):
            a = k * TILE_N
            lo_w = min(TILE_N, COLS - a) if k < N_UNITS else 0
            j = k - DELAY                      # stage-2 source unit
            hi_w = min(TILE_N, COLS - j * TILE_N) if j >= 0 else 0

            zy = ps_zy.tile([128, 2 * TILE_N], F32, tag="zy", name="zy")

            if lo_w:
                kb = k // OB
                if itile_cur[1] != kb:
                    span = min(OB * TILE_N, COLS - kb * OB * TILE_N)
                    itile = p_inp.tile([IN_K, OB * TILE_N], BF16, tag="inp",
                                       name="inp")
                    nc.sync.dma_start(
                        itile[:, 0:span],
                        inp_ap[:, kb * OB * TILE_N:kb * OB * TILE_N + span])
                    itile_cur[0] = itile
                    itile_cur[1] = kb
                itile = itile_cur[0]
                ioff = a - kb * OB * TILE_N
                for c0 in range(a, a + lo_w, SUB):
                    s = seg_of(c0)
                    nc.tensor.matmul(
                        zy[:, c0 - a:c0 - a + SUB],
                        lhsT=wseg[:, s * 128:(s + 1) * 128],
                        rhs=itile[:, ioff + c0 - a:ioff + c0 - a + SUB],
                        start=True, stop=True)

            if hi_w:
                t1 = t1_live[j]
                for c0 in range(0, hi_w, SUB):
                    nc.tensor.matmul(
                        zy[:, TILE_N + c0:TILE_N + c0 + SUB],
                        lhsT=m2[:, :],
                        rhs=t1[:, c0:c0 + SUB],
                        start=True, stop=True)

            # NOTE: exp/DVE are emitted per half (not one 2048-wide op):
            # with only 2 PSUM buffers the WAR loop consumer(k-2)->producer(k)
            # must stay shorter than the Vector issue rate, and 2048-wide
            # serial exp->dve chains violate that (measured +113us).
            ot = p_ot.tile([128, 2 * TILE_N], BF16, tag="ot", name="ot")
            if lo_w:
                elu_ops(zy, ot, 0, lo_w)
            if hi_w:
                elu_ops(zy, ot, TILE_N, hi_w)

            if lo_w:
                t1_live[k] = ot
            if hi_w:
                nc.sync.dma_start(out_ap[:, j * TILE_N:j * TILE_N + hi_w],
                                  ot[:, TILE_N:TILE_N + hi_w])
                del t1_live[j]


def _prep_inputs(x_SB, c_SB, x_PQ, c_PQ, x_PV, c_PV, x_NB, c_NB,
                 W_fc, b_fc, W2, b2):
    bf = ml_dtypes.bfloat16
    xs = {"SB": x_SB, "PQ": x_PQ, "PV": x_PV, "NB": x_NB}
    cs = {"SB": c_SB, "PQ": c_PQ, "PV": c_PV, "NB": c_NB}

    w2f = W2.astype(np.float32)
    # fused stage-2 weight: y[c,n] = sum_i m2[i,c] * t1[i,n]
    m2 = np.zeros((128, 128), dtype=bf)
    m2[:64, :] = w2f[0][None, :].astype(bf)
    m2[64:, :] = w2f[1][None, :].astype(bf)
    # the matmul of t1 = h+1 adds exactly sum_i m2[i,c] (the ROUNDED
    # weights); correct with that same rounded sum, not the exact one
    b2adj_v = (b2.astype(np.float32) - m2.astype(np.float32).sum(axis=0))
    b2adj = b2adj_v.reshape(128, 1)
    b2adjp1 = b2adj + 1.0

    # per-segment weights with bias fold: rows 11/12 carry a hi/lo bf16
    # split of (b_fc[l,t] - b2adj) so z' = z + b_fc - b2adj
    wseg = np.zeros((IN_K, NUM_LAYERS * 4 * 128), dtype=bf)
    for ti in range(4):
        for l in range(NUM_LAYERS):
            s = ti * NUM_LAYERS + l
            blk = np.zeros((IN_K, 128), dtype=np.float32)
            blk[:11] = W_fc[l, ti]
            bias = b_fc[l, ti].astype(np.float32) - b2adj_v
            hi = bias.astype(bf).astype(np.float32)
            blk[11] = hi
            blk[12] = bias - hi
            wseg[:, s * 128:(s + 1) * 128] = blk.astype(bf)

    # concatenated per-core input stream
    inp_cat = np.zeros((N_CORES, IN_K, COLS), dtype=bf)
    c = 0
    for t in NODE_TYPES:
        xT = xs[t].T.astype(bf)
        cT = cs[t].T.astype(bf)
        v = VPC[t]
        blk = np.zeros((N_CORES, IN_K, PPC[t]), dtype=bf)
        for i in range(N_CORES):
            blk[i, :4, :v] = xT[:, i * v:(i + 1) * v]
            blk[i, 4:11, :v] = cT[:, i * v:(i + 1) * v]
        blk[:, 11:13, :] = 1.0
        for l in range(NUM_LAYERS):
            inp_cat[:, :, c:c + PPC[t]] = blk
            c += PPC[t]
    assert c == COLS

    in_maps = []
    for i in range(N_CORES):
        in_maps.append(dict(inp_cat=inp_cat[i], wseg=wseg, m2=m2,
                            b2adj=b2adj, b2adjp1=b2adjp1))
    return in_maps


def kernel(**inputs):
    if "nc" not in _CACHE:
        _CACHE["nc"] = _build_nc()
    nc = _CACHE["nc"]
    in_maps = _prep_inputs(**inputs)
    trace = bool(int(os.environ.get("K_TRACE", "0")))
    res = run_bass_kernel_spmd(nc, in_maps, core_ids=list(range(N_CORES)),
                               trace=trace)
    _CACHE["last_result"] = res
    outs = res.results if hasattr(res, "results") else res

    full = np.empty((NUM_LAYERS * sum(SIZES.values()), 128), dtype=np.float32)
    row = 0
    type_row0 = {}
    for t in NODE_TYPES:
        type_row0[t] = row
        row += NUM_LAYERS * SIZES[t]
    for i in range(N_CORES):
        o = np.asarray(outs[i]["out"])           # [128, COLS] bf16
        oT = o.T.astype(np.float32) - 1.0        # out stored as ELU+1
        base = 0
        for t in NODE_TYPES:
            for l in range(NUM_LAYERS):
                src = base + l * PPC[t]
                dst = type_row0[t] + l * SIZES[t] + i * VPC[t]
                full[dst:dst + VPC[t]] = oT[src:src + VPC[t]]
            base += NUM_LAYERS * PPC[t]
    return full


# revision 23
# speedup vs baseline: 1.5093x; 1.5093x over previous
"""Trainium2 Bass kernel for nn_ACOPFEnforcer (per-node-type MLP, no message passing).

Math per node type t (sizes SB=4000, PQ=200000, PV=80000, NB=116000):
    inp = concat(x_t, c_t)                      # [N, 11]
    z_l = inp @ W[l,t] + b[l,t]                 # l = 0..2, [N, 128]
    h_l = ELU(z_l)
    P_l = h_l[:, :64].sum(-1); Q_l = h_l[:, 64:].sum(-1)
    out[l*N+n] = ELU(P_l[n]*W2[0] + Q_l[n]*W2[1] + b2)   # [128]
Output = concat over types of the [3*N_t, 128] blocks.

Strategy: pure data parallelism over nodes across 8 NeuronCores, channels
on partitions / nodes on the free dim.

Key fusions:
- Stage-2 y[c,n] = w0[c]*P[n] + w1[c]*Q[n] = sum_i M[i,c]*t1[i,n] with
  M[i,c] = W2[0,c] (i<64) / W2[1,c] (i>=64): the P/Q half-sums and the
  2->128 linear collapse into one K=128 matmul (the +1 shift of
  t1 = ELU+1 folds into b2adj = b2 - sum_i M[i,c]).
- ELU(z)+1 = max(z+b+1, min(exp(z+b), 1)): one ScalarE Exp + ONE fused
  custom-DVE op (registered at import via the documented dve_ops
  extension point) per tile.
- Stage-1 per-segment bias is folded into the matmul via two constant-one
  input rows carrying a hi/lo bf16 split of (b_fc - b2adj), so the Exp
  bias (b2adj) and the DVE scalar (b2adj+1) are GLOBAL constants.  That
  makes every elementwise instruction identical across the whole kernel,
  allowing stage-1 z of unit k and stage-2 y of unit k-D to share one
  [128, 2048] PSUM tile and be processed by single 2048-wide Exp/DVE
  instructions (halving per-instruction overheads).
- All (type, layer) column segments concatenate into one per-core column
  stream at 512-column granularity, so padding waste is ~0.5% instead of
  4% and there is exactly one partial tile per core.

Output is written bf16 in [channel, node] layout (values stored as ELU+1)
and transposed/cast/-1 on the host.
"""

import os
import numpy as np
import ml_dtypes

import concourse.bass as bass
import concourse.tile as tile
from concourse import bacc, mybir
from concourse.bass_utils import run_bass_kernel_spmd

BF16 = mybir.dt.bfloat16
F32 = mybir.dt.float32
AF = mybir.ActivationFunctionType
OP = mybir.AluOpType

NODE_TYPES = ["SB", "PQ", "PV", "NB"]
SIZES = {"SB": 4000, "PQ": 200000, "PV": 80000, "NB": 116000}
NUM_LAYERS = 3
N_CORES = 8
TILE_N = 1024          # stream unit width (z half of the psum tile)
SUB = 512              # PSUM bank width in fp32 == matmul ISA width cap
DELAY = int(os.environ.get("K_D", "4"))   # units between stage-1 and stage-2
IN_K = 13              # 4 x + 7 c + 2 ones (bias hi/lo) input rows

# per-core per-type padded sizes at 512 granularity
PPC = {t: -(-SIZES[t] // (N_CORES * SUB)) * SUB for t in NODE_TYPES}
VPC = {t: SIZES[t] // N_CORES for t in NODE_TYPES}      # valid nodes/core
COLS = NUM_LAYERS * sum(PPC.values())                   # stream cols/core
N_UNITS = -(-COLS // TILE_N)

_CACHE = {}


def _register_elu_op():
    """Register the fused ELU combine as a custom DVE op:
    out = max(in0 + s0, min(in1, 1))   [in0: z psum f32, in1: exp(z+b) bf16]
    This is ELU(z+b)+1 when s0 = b+1 and in1 = exp(z+b)."""
    if "elu_op" in _CACHE:
        return _CACHE["elu_op"]
    import concourse.dve_ops as dve_ops
    from concourse.dve_spec import Spec, Src0, Src1, C0, One, maxx, minn, lower
    from concourse.dve_uop import DveOpSpec

    name = "ELU_SHIFT_COMBINE_ANT"
    body = maxx(Src0 + C0, minn(Src1, One))
    spec = Spec(
        body=body,
        reference=lambda in0, in1, c0, c1, c2: np.maximum(
            np.asarray(in0, np.float32) + c0,
            np.minimum(np.asarray(in1, np.float32), 1.0)),
    )
    if name not in dve_ops._SUB_OPCODE_FOR_NAME:
        row = max(dve_ops._SUB_OPCODE_FOR_NAME.values()) + 1
        assert row < 0x20
        dve_ops._SUB_OPCODE_FOR_NAME[name] = row
    # pin the sha the same way DveOp.compile derives it
    shas = {}
    for ver in ("v3", "v4"):
        s = DveOpSpec(name=name, opcode=dve_ops._SUB_OPCODE_FOR_NAME[name],
                      uops=lower(spec, ver=ver), rd1_en=True)
        shas[ver] = s.sha(ver)
    op = dve_ops.DveOp(name, spec, subdim=False, uops_sha=shas)
    if not any(o.name == name for o in dve_ops.OPS):
        dve_ops.OPS.append(op)
    dve_ops.CUSTOM_DVE_SPECS[name] = spec
    _CACHE["elu_op"] = op
    return op


def _segments():
    """[(seg_idx, start_col, end_col)] of the per-core column stream."""
    segs = []
    c = 0
    for ti, t in enumerate(NODE_TYPES):
        for l in range(NUM_LAYERS):
            segs.append((ti * NUM_LAYERS + l, c, c + PPC[t]))
            c += PPC[t]
    assert c == COLS
    return segs


def _build_nc():
    _register_elu_op()
    nc = bacc.Bacc("TRN2", target_bir_lowering=False, debug=False,
                   enable_asserts=False, num_devices=N_CORES)

    inp_ap = nc.dram_tensor("inp_cat", [IN_K, COLS], BF16,
                            kind="ExternalInput").ap()
    wseg_ap = nc.dram_tensor("wseg", [IN_K, NUM_LAYERS * 4 * 128], BF16,
                             kind="ExternalInput").ap()
    m2_ap = nc.dram_tensor("m2", [128, 128], BF16, kind="ExternalInput").ap()
    b2adj_ap = nc.dram_tensor("b2adj", [128, 1], F32, kind="ExternalInput").ap()
    b2adjp1_ap = nc.dram_tensor("b2adjp1", [128, 1], F32,
                                kind="ExternalInput").ap()
    out_ap = nc.dram_tensor("out", [128, COLS], BF16,
                            kind="ExternalOutput").ap()

    with tile.TileContext(nc) as tc:
        _emit(tc, inp_ap, wseg_ap, m2_ap, b2adj_ap, b2adjp1_ap, out_ap)
    nc.compile()
    return nc


def _emit(tc, inp_ap, wseg_ap, m2_ap, b2adj_ap, b2adjp1_ap, out_ap):
    nc = tc.nc
    from contextlib import ExitStack
    ctx = ExitStack()
    with ctx:
        elu_op = _CACHE["elu_op"]
        consts = ctx.enter_context(tc.tile_pool(name="consts", bufs=1))
        p_inp = ctx.enter_context(tc.tile_pool(name="inp", bufs=3))
        p_e = ctx.enter_context(tc.tile_pool(name="e", bufs=3))
        p_ot = ctx.enter_context(tc.tile_pool(name="ot", bufs=DELAY + 3))
        ps_z = ctx.enter_context(tc.tile_pool(name="zps", bufs=2,
                                              space="PSUM"))
        ps_y = ctx.enter_context(tc.tile_pool(name="yps", bufs=2,
                                              space="PSUM"))

        wseg = consts.tile([IN_K, NUM_LAYERS * 4 * 128], BF16, tag="wseg",
                           name="wseg")
        nc.sync.dma_start(wseg[:], wseg_ap[:])
        m2 = consts.tile([128, 128], BF16, tag="m2", name="m2")
        nc.sync.dma_start(m2[:], m2_ap[:])
        b2adj = consts.tile([128, 1], F32, tag="b2adj", name="b2adj")
        nc.sync.dma_start(b2adj[:], b2adj_ap[:])
        b2adjp1 = consts.tile([128, 1], F32, tag="b2adjp1", name="b2adjp1")
        nc.sync.dma_start(b2adjp1[:], b2adjp1_ap[:])

        segs = _segments()

        def seg_of(col):
            for s, a, b in segs:
                if a <= col < b:
                    return s
            raise AssertionError(col)

        OB = 4   # units per input DMA batch
        t1_live = {}   # unit -> out tile (lo half is t1)
        itile_cur = [None, -1]   # tile, batch idx

        def elu_ops(src, ot, off, w):
            e = p_e.tile([128, TILE_N], BF16, tag="e", name="e")
            nc.scalar.activation(e[:, 0:w], src[:, 0:w],
                                 AF.Exp, bias=b2adj[:, 0:1])
            nc.vector._custom_dve(elu_op, out=ot[:, off:off + w],
                                  in0=src[:, 0:w],
                                  in1=e[:, 0:w],
                                  s0=b2adjp1[:, 0:1])

        for k in range(N_UNITS + DELAY):
            a = k * TILE_N
            lo_w = min(TILE_N, COLS - a) if k < N_UNITS else 0
            j = k - DELAY                      # stage-2 source unit
            hi_w = min(TILE_N, COLS - j * TILE_N) if j >= 0 else 0

            if lo_w:
                z = ps_z.tile([128, TILE_N], F32, tag="zps", name="zps")
                kb = k // OB
                if itile_cur[1] != kb:
                    span = min(OB * TILE_N, COLS - kb * OB * TILE_N)
                    itile = p_inp.tile([IN_K, OB * TILE_N], BF16, tag="inp",
                                       name="inp")
                    nc.sync.dma_start(
                        itile[:, 0:span],
                        inp_ap[:, kb * OB * TILE_N:kb * OB * TILE_N + span])
                    itile_cur[0] = itile
                    itile_cur[1] = kb
                itile = itile_cur[0]
                ioff = a - kb * OB * TILE_N
                for c0 in range(a, a + lo_w, SUB):
                    s = seg_of(c0)
                    nc.tensor.matmul(
                        z[:, c0 - a:c0 - a + SUB],
                        lhsT=wseg[:, s * 128:(s + 1) * 128],
                        rhs=itile[:, ioff + c0 - a:ioff + c0 - a + SUB],
                        start=True, stop=True)

            if hi_w:
                y = ps_y.tile([128, TILE_N], F32, tag="yps", name="yps")
                t1 = t1_live[j]
                for c0 in range(0, hi_w, SUB):
                    nc.tensor.matmul(
                        y[:, c0:c0 + SUB],
                        lhsT=m2[:, :],
                        rhs=t1[:, c0:c0 + SUB],
                        start=True, stop=True)

            ot = p_ot.tile([128, 2 * TILE_N], BF16, tag="ot", name="ot")
            if lo_w:
                elu_ops(z, ot, 0, lo_w)
            if hi_w:
                elu_ops(y, ot, TILE_N, hi_w)

            if lo_w:
                t1_live[k] = ot
            if hi_w:
                nc.sync.dma_start(out_ap[:, j * TILE_N:j * TILE_N + hi_w],
                                  ot[:, TILE_N:TILE_N + hi_w])
                del t1_live[j]


def _prep_inputs(x_SB, c_SB, x_PQ, c_PQ, x_PV, c_PV, x_NB, c_NB,
                 W_fc, b_fc, W2, b2):
    bf = ml_dtypes.bfloat16
    xs = {"SB": x_SB, "PQ": x_PQ, "PV": x_PV, "NB": x_NB}
    cs = {"SB": c_SB, "PQ": c_PQ, "PV": c_PV, "NB": c_NB}

    w2f = W2.astype(np.float32)
    # fused stage-2 weight: y[c,n] = sum_i m2[i,c] * t1[i,n]
    m2 = np.zeros((128, 128), dtype=bf)
    m2[:64, :] = w2f[0][None, :].astype(bf)
    m2[64:, :] = w2f[1][None, :].astype(bf)
    # the matmul of t1 = h+1 adds exactly sum_i m2[i,c] (the ROUNDED
    # weights); correct with that same rounded sum, not the exact one
    b2adj_v = (b2.astype(np.float32) - m2.astype(np.float32).sum(axis=0))
    b2adj = b2adj_v.reshape(128, 1)
    b2adjp1 = b2adj + 1.0

    # per-segment weights with bias fold: rows 11/12 carry a hi/lo bf16
    # split of (b_fc[l,t] - b2adj) so z' = z + b_fc - b2adj
    wseg = np.zeros((IN_K, NUM_LAYERS * 4 * 128), dtype=bf)
    for ti in range(4):
        for l in range(NUM_LAYERS):
            s = ti * NUM_LAYERS + l
            blk = np.zeros((IN_K, 128), dtype=np.float32)
            blk[:11] = W_fc[l, ti]
            bias = b_fc[l, ti].astype(np.float32) - b2adj_v
            hi = bias.astype(bf).astype(np.float32)
            blk[11] = hi
            blk[12] = bias - hi
            wseg[:, s * 128:(s + 1) * 128] = blk.astype(bf)

    # concatenated per-core input stream
    inp_cat = np.zeros((N_CORES, IN_K, COLS), dtype=bf)
    c = 0
    for t in NODE_TYPES:
        xT = xs[t].T.astype(bf)
        cT = cs[t].T.astype(bf)
        v = VPC[t]
        blk = np.zeros((N_CORES, IN_K, PPC[t]), dtype=bf)
        for i in range(N_CORES):
            blk[i, :4, :v] = xT[:, i * v:(i + 1) * v]
            blk[i, 4:11, :v] = cT[:, i * v:(i + 1) * v]
        blk[:, 11:13, :] = 1.0
        for l in range(NUM_LAYERS):
            inp_cat[:, :, c:c + PPC[t]] = blk
            c += PPC[t]
    assert c == COLS

    in_maps = []
    for i in range(N_CORES):
        in_maps.append(dict(inp_cat=inp_cat[i], wseg=wseg, m2=m2,
                            b2adj=b2adj, b2adjp1=b2adjp1))
    return in_maps


def kernel(**inputs):
    if "nc" not in _CACHE:
        _CACHE["nc"] = _build_nc()
    nc = _CACHE["nc"]
    in_maps = _prep_inputs(**inputs)
    trace = bool(int(os.environ.get("K_TRACE", "0")))
    res = run_bass_kernel_spmd(nc, in_maps, core_ids=list(range(N_CORES)),
                               trace=trace)
    _CACHE["last_result"] = res
    outs = res.results if hasattr(res, "results") else res

    full = np.empty((NUM_LAYERS * sum(SIZES.values()), 128), dtype=np.float32)
    row = 0
    type_row0 = {}
    for t in NODE_TYPES:
        type_row0[t] = row
        row += NUM_LAYERS * SIZES[t]
    for i in range(N_CORES):
        o = np.asarray(outs[i]["out"])           # [128, COLS] bf16
        oT = o.T.astype(np.float32) - 1.0        # out stored as ELU+1
        base = 0
        for t in NODE_TYPES:
            for l in range(NUM_LAYERS):
                src = base + l * PPC[t]
                dst = type_row0[t] + l * SIZES[t] + i * VPC[t]
                full[dst:dst + VPC[t]] = oT[src:src + VPC[t]]
            base += NUM_LAYERS * PPC[t]
    return full


# revision 27
# speedup vs baseline: 1.5119x; 1.0017x over previous
"""Trainium2 Bass kernel for nn_ACOPFEnforcer (per-node-type MLP, no message passing).

Math per node type t (sizes SB=4000, PQ=200000, PV=80000, NB=116000):
    inp = concat(x_t, c_t)                      # [N, 11]
    z_l = inp @ W[l,t] + b[l,t]                 # l = 0..2, [N, 128]
    h_l = ELU(z_l)
    P_l = h_l[:, :64].sum(-1); Q_l = h_l[:, 64:].sum(-1)
    out[l*N+n] = ELU(P_l[n]*W2[0] + Q_l[n]*W2[1] + b2)   # [128]
Output = concat over types of the [3*N_t, 128] blocks.

Strategy: pure data parallelism over nodes across 8 NeuronCores, channels
on partitions / nodes on the free dim.

Key fusions:
- Stage-2 y[c,n] = w0[c]*P[n] + w1[c]*Q[n] = sum_i M[i,c]*t1[i,n] with
  M[i,c] = W2[0,c] (i<64) / W2[1,c] (i>=64): the P/Q half-sums and the
  2->128 linear collapse into one K=128 matmul (the +1 shift of
  t1 = ELU+1 folds into b2adj = b2 - sum_i M[i,c]).
- ELU(z)+1 = max(z+b+1, min(exp(z+b), 1)): one ScalarE Exp + ONE fused
  custom-DVE op (registered at import via the documented dve_ops
  extension point) per tile.
- Stage-1 per-segment bias is folded into the matmul via two constant-one
  input rows carrying a hi/lo bf16 split of (b_fc - b2adj), so the Exp
  bias (b2adj) and the DVE scalar (b2adj+1) are GLOBAL constants.  That
  makes every elementwise instruction identical across the whole kernel,
  allowing stage-1 z of unit k and stage-2 y of unit k-D to share one
  [128, 2048] PSUM tile and be processed by single 2048-wide Exp/DVE
  instructions (halving per-instruction overheads).
- All (type, layer) column segments concatenate into one per-core column
  stream at 512-column granularity, so padding waste is ~0.5% instead of
  4% and there is exactly one partial tile per core.

Output is written bf16 in [channel, node] layout (values stored as ELU+1)
and transposed/cast/-1 on the host.
"""

import os
import numpy as np
import ml_dtypes

import concourse.bass as bass
import concourse.tile as tile
from concourse import bacc, mybir
from concourse.bass_utils import run_bass_kernel_spmd

BF16 = mybir.dt.bfloat16
F32 = mybir.dt.float32
AF = mybir.ActivationFunctionType
OP = mybir.AluOpType

NODE_TYPES = ["SB", "PQ", "PV", "NB"]
SIZES = {"SB": 4000, "PQ": 200000, "PV": 80000, "NB": 116000}
NUM_LAYERS = 3
N_CORES = 8
TILE_N = 1024          # stream unit width (z half of the psum tile)
SUB = 512              # PSUM bank width in fp32 == matmul ISA width cap
DELAY = int(os.environ.get("K_D", "4"))   # units between stage-1 and stage-2
C2_FRAC = float(os.environ.get("K_C2", "0.10"))  # ScalarE-relu path share
IN_K = 13              # 4 x + 7 c + 2 ones (bias hi/lo) input rows

# per-core per-type padded sizes at 512 granularity
PPC = {t: -(-SIZES[t] // (N_CORES * SUB)) * SUB for t in NODE_TYPES}
VPC = {t: SIZES[t] // N_CORES for t in NODE_TYPES}      # valid nodes/core
COLS = NUM_LAYERS * sum(PPC.values())                   # stream cols/core
N_UNITS = -(-COLS // TILE_N)

_CACHE = {}


def _register_elu_op():
    """Register the fused ELU combine as a custom DVE op:
    out = max(in0 + s0, min(in1, 1))   [in0: z psum f32, in1: exp(z+b) bf16]
    This is ELU(z+b)+1 when s0 = b+1 and in1 = exp(z+b)."""
    if "elu_op" in _CACHE:
        return _CACHE["elu_op"]
    import concourse.dve_ops as dve_ops
    from concourse.dve_spec import Spec, Src0, Src1, C0, One, maxx, minn, lower
    from concourse.dve_uop import DveOpSpec

    name = "ELU_SHIFT_COMBINE_ANT"
    body = maxx(Src0 + C0, minn(Src1, One))
    spec = Spec(
        body=body,
        reference=lambda in0, in1, c0, c1, c2: np.maximum(
            np.asarray(in0, np.float32) + c0,
            np.minimum(np.asarray(in1, np.float32), 1.0)),
    )
    if name not in dve_ops._SUB_OPCODE_FOR_NAME:
        row = max(dve_ops._SUB_OPCODE_FOR_NAME.values()) + 1
        assert row < 0x20
        dve_ops._SUB_OPCODE_FOR_NAME[name] = row
    # pin the sha the same way DveOp.compile derives it
    shas = {}
    for ver in ("v3", "v4"):
        s = DveOpSpec(name=name, opcode=dve_ops._SUB_OPCODE_FOR_NAME[name],
                      uops=lower(spec, ver=ver), rd1_en=True)
        shas[ver] = s.sha(ver)
    op = dve_ops.DveOp(name, spec, subdim=False, uops_sha=shas)
    if not any(o.name == name for o in dve_ops.OPS):
        dve_ops.OPS.append(op)
    dve_ops.CUSTOM_DVE_SPECS[name] = spec
    _CACHE["elu_op"] = op

    # second variant for the ScalarE-relu path: out = min(in0 + 1, in1)
    # (= min(relu(z+b)+1, exp(z+b)) = ELU+1).  All-bf16-SBUF streams, so
    # opt into the DVE 2x perf-mode table slots.
    name2 = "ELU_RELU_COMBINE_ANT"
    spec2 = Spec(
        body=minn(Src0 + One, Src1),
        reference=lambda in0, in1, c0, c1, c2: np.minimum(
            np.asarray(in0, np.float32) + 1.0,
            np.asarray(in1, np.float32)),
    )
    if name2 not in dve_ops._SUB_OPCODE_FOR_NAME:
        dve_ops._SUB_OPCODE_FOR_NAME[name2] = \
            max(dve_ops._SUB_OPCODE_FOR_NAME.values()) + 1
    shas2 = {}
    for ver in ("v3", "v4"):
        s = DveOpSpec(name=name2, opcode=dve_ops._SUB_OPCODE_FOR_NAME[name2],
                      uops=lower(spec2, ver=ver), rd1_en=True)
        shas2[ver] = s.sha(ver)
    op2 = dve_ops.DveOp(name2, spec2, subdim=False, uops_sha=shas2,
                        perf_en={"v3": True, "v4": True})
    if not any(o.name == name2 for o in dve_ops.OPS):
        dve_ops.OPS.append(op2)
    dve_ops.CUSTOM_DVE_SPECS[name2] = spec2
    _CACHE["elu2_op"] = op2
    return op


def _segments():
    """[(seg_idx, start_col, end_col)] of the per-core column stream."""
    segs = []
    c = 0
    for ti, t in enumerate(NODE_TYPES):
        for l in range(NUM_LAYERS):
            segs.append((ti * NUM_LAYERS + l, c, c + PPC[t]))
            c += PPC[t]
    assert c == COLS
    return segs


def _build_nc():
    _register_elu_op()
    nc = bacc.Bacc("TRN2", target_bir_lowering=False, debug=False,
                   enable_asserts=False, num_devices=N_CORES)

    inp_ap = nc.dram_tensor("inp_cat", [IN_K, COLS], BF16,
                            kind="ExternalInput").ap()
    wseg_ap = nc.dram_tensor("wseg", [IN_K, NUM_LAYERS * 4 * 128], BF16,
                             kind="ExternalInput").ap()
    m2_ap = nc.dram_tensor("m2", [128, 128], BF16, kind="ExternalInput").ap()
    b2adj_ap = nc.dram_tensor("b2adj", [128, 1], F32, kind="ExternalInput").ap()
    b2adjp1_ap = nc.dram_tensor("b2adjp1", [128, 1], F32,
                                kind="ExternalInput").ap()
    out_ap = nc.dram_tensor("out", [128, COLS], BF16,
                            kind="ExternalOutput").ap()

    with tile.TileContext(nc) as tc:
        _emit(tc, inp_ap, wseg_ap, m2_ap, b2adj_ap, b2adjp1_ap, out_ap)
    nc.compile()
    return nc


def _emit(tc, inp_ap, wseg_ap, m2_ap, b2adj_ap, b2adjp1_ap, out_ap):
    nc = tc.nc
    from contextlib import ExitStack
    ctx = ExitStack()
    with ctx:
        elu_op = _CACHE["elu_op"]
        consts = ctx.enter_context(tc.tile_pool(name="consts", bufs=1))
        p_inp = ctx.enter_context(tc.tile_pool(name="inp", bufs=3))
        p_e = ctx.enter_context(tc.tile_pool(name="e", bufs=3))
        p_r = ctx.enter_context(tc.tile_pool(name="r", bufs=3))
        p_ot = ctx.enter_context(tc.tile_pool(name="ot", bufs=DELAY + 3))
        ps_z = ctx.enter_context(tc.tile_pool(name="zps", bufs=2,
                                              space="PSUM"))
        ps_y = ctx.enter_context(tc.tile_pool(name="yps", bufs=2,
                                              space="PSUM"))

        wseg = consts.tile([IN_K, NUM_LAYERS * 4 * 128], BF16, tag="wseg",
                           name="wseg")
        nc.sync.dma_start(wseg[:], wseg_ap[:])
        m2 = consts.tile([128, 128], BF16, tag="m2", name="m2")
        nc.sync.dma_start(m2[:], m2_ap[:])
        b2adj = consts.tile([128, 1], F32, tag="b2adj", name="b2adj")
        nc.sync.dma_start(b2adj[:], b2adj_ap[:])
        b2adjp1 = consts.tile([128, 1], F32, tag="b2adjp1", name="b2adjp1")
        nc.sync.dma_start(b2adjp1[:], b2adjp1_ap[:])

        segs = _segments()

        def seg_of(col):
            for s, a, b in segs:
                if a <= col < b:
                    return s
            raise AssertionError(col)

        OB = 4   # units per input DMA batch
        t1_live = {}   # unit -> out tile (lo half is t1)
        itile_cur = [None, -1]   # tile, batch idx

        elu2_op = _CACHE["elu2_op"]
        cnt = [0]

        def elu_ops(src, ot, off, w):
            e = p_e.tile([128, TILE_N], BF16, tag="e", name="e")
            nc.scalar.activation(e[:, 0:w], src[:, 0:w],
                                 AF.Exp, bias=b2adj[:, 0:1])
            cnt[0] += 1
            if (cnt[0] * 7) % 100 < C2_FRAC * 100:
                # ScalarE-relu path: frees the Vector op from the PSUM
                # stream so it can run in 2x mode (all-bf16 SBUF)
                r = p_r.tile([128, TILE_N], BF16, tag="r", name="r")
                nc.scalar.activation(r[:, 0:w], src[:, 0:w],
                                     AF.Relu, bias=b2adj[:, 0:1])
                nc.vector._custom_dve(elu2_op, out=ot[:, off:off + w],
                                      in0=r[:, 0:w], in1=e[:, 0:w])
            else:
                nc.vector._custom_dve(elu_op, out=ot[:, off:off + w],
                                      in0=src[:, 0:w],
                                      in1=e[:, 0:w],
                                      s0=b2adjp1[:, 0:1])

        for k in range(N_UNITS + DELAY):
            a = k * TILE_N
            lo_w = min(TILE_N, COLS - a) if k < N_UNITS else 0
            j = k - DELAY                      # stage-2 source unit
            hi_w = min(TILE_N, COLS - j * TILE_N) if j >= 0 else 0

            if lo_w:
                z = ps_z.tile([128, TILE_N], F32, tag="zps", name="zps")
                kb = k // OB
                if itile_cur[1] != kb:
                    span = min(OB * TILE_N, COLS - kb * OB * TILE_N)
                    itile = p_inp.tile([IN_K, OB * TILE_N], BF16, tag="inp",
                                       name="inp")
                    nc.sync.dma_start(
                        itile[:, 0:span],
                        inp_ap[:, kb * OB * TILE_N:kb * OB * TILE_N + span])
                    itile_cur[0] = itile
                    itile_cur[1] = kb
                itile = itile_cur[0]
                ioff = a - kb * OB * TILE_N
                for c0 in range(a, a + lo_w, SUB):
                    s = seg_of(c0)
                    nc.tensor.matmul(
                        z[:, c0 - a:c0 - a + SUB],
                        lhsT=wseg[:, s * 128:(s + 1) * 128],
                        rhs=itile[:, ioff + c0 - a:ioff + c0 - a + SUB],
                        start=True, stop=True)

            if hi_w:
                y = ps_y.tile([128, TILE_N], F32, tag="yps", name="yps")
                t1 = t1_live[j]
                for c0 in range(0, hi_w, SUB):
                    nc.tensor.matmul(
                        y[:, c0:c0 + SUB],
                        lhsT=m2[:, :],
                        rhs=t1[:, c0:c0 + SUB],
                        start=True, stop=True)

            ot = p_ot.tile([128, 2 * TILE_N], BF16, tag="ot", name="ot")
            if lo_w:
                elu_ops(z, ot, 0, lo_w)
            if hi_w:
                elu_ops(y, ot, TILE_N, hi_w)

            if lo_w:
                t1_live[k] = ot
            if hi_w:
                nc.sync.dma_start(out_ap[:, j * TILE_N:j * TILE_N + hi_w],
                                  ot[:, TILE_N:TILE_N + hi_w])
                del t1_live[j]


def _prep_inputs(x_SB, c_SB, x_PQ, c_PQ, x_PV, c_PV, x_NB, c_NB,
                 W_fc, b_fc, W2, b2):
    bf = ml_dtypes.bfloat16
    xs = {"SB": x_SB, "PQ": x_PQ, "PV": x_PV, "NB": x_NB}
    cs = {"SB": c_SB, "PQ": c_PQ, "PV": c_PV, "NB": c_NB}

    w2f = W2.astype(np.float32)
    # fused stage-2 weight: y[c,n] = sum_i m2[i,c] * t1[i,n]
    m2 = np.zeros((128, 128), dtype=bf)
    m2[:64, :] = w2f[0][None, :].astype(bf)
    m2[64:, :] = w2f[1][None, :].astype(bf)
    # the matmul of t1 = h+1 adds exactly sum_i m2[i,c] (the ROUNDED
    # weights); correct with that same rounded sum, not the exact one
    b2adj_v = (b2.astype(np.float32) - m2.astype(np.float32).sum(axis=0))
    b2adj = b2adj_v.reshape(128, 1)
    b2adjp1 = b2adj + 1.0

    # per-segment weights with bias fold: rows 11/12 carry a hi/lo bf16
    # split of (b_fc[l,t] - b2adj) so z' = z + b_fc - b2adj
    wseg = np.zeros((IN_K, NUM_LAYERS * 4 * 128), dtype=bf)
    for ti in range(4):
        for l in range(NUM_LAYERS):
            s = ti * NUM_LAYERS + l
            blk = np.zeros((IN_K, 128), dtype=np.float32)
            blk[:11] = W_fc[l, ti]
            bias = b_fc[l, ti].astype(np.float32) - b2adj_v
            hi = bias.astype(bf).astype(np.float32)
            blk[11] = hi
            blk[12] = bias - hi
            wseg[:, s * 128:(s + 1) * 128] = blk.astype(bf)

    # concatenated per-core input stream
    inp_cat = np.zeros((N_CORES, IN_K, COLS), dtype=bf)
    c = 0
    for t in NODE_TYPES:
        xT = xs[t].T.astype(bf)
        cT = cs[t].T.astype(bf)
        v = VPC[t]
        blk = np.zeros((N_CORES, IN_K, PPC[t]), dtype=bf)
        for i in range(N_CORES):
            blk[i, :4, :v] = xT[:, i * v:(i + 1) * v]
            blk[i, 4:11, :v] = cT[:, i * v:(i + 1) * v]
        blk[:, 11:13, :] = 1.0
        for l in range(NUM_LAYERS):
            inp_cat[:, :, c:c + PPC[t]] = blk
            c += PPC[t]
    assert c == COLS

    in_maps = []
    for i in range(N_CORES):
        in_maps.append(dict(inp_cat=inp_cat[i], wseg=wseg, m2=m2,
                            b2adj=b2adj, b2adjp1=b2adjp1))
    return in_maps


def kernel(**inputs):
    if "nc" not in _CACHE:
        _CACHE["nc"] = _build_nc()
    nc = _CACHE["nc"]
    in_maps = _prep_inputs(**inputs)
    trace = bool(int(os.environ.get("K_TRACE", "0")))
    res = run_bass_kernel_spmd(nc, in_maps, core_ids=list(range(N_CORES)),
                               trace=trace)
    _CACHE["last_result"] = res
    outs = res.results if hasattr(res, "results") else res

    full = np.empty((NUM_LAYERS * sum(SIZES.values()), 128), dtype=np.float32)
    row = 0
    type_row0 = {}
    for t in NODE_TYPES:
        type_row0[t] = row
        row += NUM_LAYERS * SIZES[t]
    for i in range(N_CORES):
        o = np.asarray(outs[i]["out"])           # [128, COLS] bf16
        oT = o.T.astype(np.float32) - 1.0        # out stored as ELU+1
        base = 0
        for t in NODE_TYPES:
            for l in range(NUM_LAYERS):
                src = base + l * PPC[t]
                dst = type_row0[t] + l * SIZES[t] + i * VPC[t]
                full[dst:dst + VPC[t]] = oT[src:src + VPC[t]]
            base += NUM_LAYERS * PPC[t]
    return full
